# revision 4
# baseline (speedup 1.0000x reference)
"""Multi-head causal attention (B=4, T=2048, E=1024, H=16) on 8 NeuronCores.

Sharding: core = (batch b, head-group g of 8 heads). Each core computes its
heads' attention + a partial output projection; host sums the two partials
per batch and adds the bias (the "all-reduce" of the tensor-parallel plan).

Device layout (per core, all fp32):
  xT  [E, T]    : x[b] transposed on host; e on partitions (8 chunks of 128).
  wq/wk/wv [E, 512]: this core's 8 heads' weights, columns = h_local*64+d.
  wpT [512, E]  : Wp rows for this core's 512 e-dims, transposed on host.
  QT/KT pair tiles [128=(2 heads' d), T]; V pair tiles [t, 128=(2 heads' d)].
  Scores computed transposed ST[t, s] (row-tiled K=64 matmul pairs), exp on
  ACT, causal diag masks via DVE mul, softmax denominator via DVE accumulate
  + ones-column matmul, PV via col-tiled matmul pairs -> OT[(h,d), s], which
  is exactly the lhsT orientation the output projection needs.
"""

import os
import sys
from contextlib import ExitStack

import numpy as np

sys.path.insert(0, "/opt/trn_rl_repo")

import concourse.bass as bass
import concourse.tile as tile
from concourse import mybir
from concourse.bass_utils import run_bass_kernel_spmd

f32 = mybir.dt.float32

B, T, E, H = 4, 2048, 1024, 16
DH = E // H          # 64
P = 128              # partitions
EC = E // P          # 8 e-chunks
NP = 4               # head pairs per core (8 heads)
SBW = 512            # s-block width
SB = T // SBW        # 4 s-blocks
TT = T // P          # 16 t-tiles
NCORES = 8

_PROG = None
LAST = {}


def _split_excess_waits(nc, limit=1):
    """walrus in this container encodes at most one sync-wait per
    instruction; move extras onto same-engine NOPs placed just before."""
    for fn in nc.m.functions:
        for bb in fn.blocks:
            out = []
            changed = False
            for inst in bb.instructions:
                si = inst.sync_info
                if si is not None and si.on_wait and len(si.on_wait) > limit:
                    waits = list(si.on_wait)
                    extra, keep = waits[:-limit], waits[-limit:]
                    for k, w in enumerate(extra):
                        nop = mybir.InstNoOp(
                            name=f"{inst.name}-wsplit{k}", ins=[], outs=[]
                        )
                        nop.engine = inst.engine
                        nop.sync_info = type(si)(on_wait=[w], on_update=[])
                        nc.register_instruction(nop)
                        out.append(nop)
                    si.on_wait = keep
                    changed = True
                out.append(inst)
            if changed:
                bb.instructions = out
    return nc


def _build_body(nc, tc, ctx, xT_d, wq_d, wk_d, wv_d, wpT_d, mask_d, y_d):
    Exp = mybir.ActivationFunctionType.Exp

    persist = ctx.enter_context(tc.tile_pool(name="persist", bufs=1))
    QT = [persist.tile([P, T], f32, tag=f"qt{p}", name=f"qt{p}") for p in range(NP)]
    KT = [persist.tile([P, T], f32, tag=f"kt{p}", name=f"kt{p}") for p in range(NP)]
    V = [persist.tile([P, TT, P], f32, tag=f"v{p}", name=f"v{p}") for p in range(NP)]
    ones_col = persist.tile([P, 1], f32, tag="ones_col")
    ones_row = persist.tile([1, DH], f32, tag="ones_row")
    masks_sb = persist.tile([P, 4, SBW], f32, tag="masks")
    nc.vector.memset(ones_col, 1.0)
    nc.vector.memset(ones_row, 1.0)
    nc.sync.dma_start(out=masks_sb, in_=mask_d.rearrange("d p m -> p d m"))

    # ---------------- Phase 1: QKV projections ----------------
    with tc.tile_pool(name="xp", bufs=1) as xp:
        xTs = xp.tile([P, EC, T], f32)
        for c in range(EC):
            nc.sync.dma_start(out=xTs[:, c, :], in_=xT_d[c * P:(c + 1) * P, :])

        # V natural [t, (2 heads' d)]
        with tc.tile_pool(name="wv", bufs=1) as wvp, \
             tc.tile_pool(name="psv", bufs=4, space="PSUM") as psv:
            wvs = wvp.tile([P, EC, NP * P], f32)
            nc.sync.dma_start(out=wvs, in_=wv_d.rearrange("(c p) m -> p c m", p=P))
            for p_ in range(NP):
                for tt in range(TT):
                    ps = psv.tile([P, P], f32)
                    for c in range(EC):
                        nc.tensor.matmul(
                            ps,
                            lhsT=xTs[:, c, tt * P:(tt + 1) * P],
                            rhs=wvs[:, c, p_ * P:(p_ + 1) * P],
                            start=(c == 0), stop=(c == EC - 1),
                        )
                    nc.vector.tensor_copy(V[p_][:, tt, :], ps)

        # QT pairs [(2 heads' d), s]
        with tc.tile_pool(name="wq", bufs=1) as wqp, \
             tc.tile_pool(name="psq", bufs=4, space="PSUM") as psq:
            wqs = wqp.tile([P, EC, NP * P], f32)
            nc.sync.dma_start(out=wqs, in_=wq_d.rearrange("(c p) m -> p c m", p=P))
            for p_ in range(NP):
                for m in range(SB):
                    ps = psq.tile([P, SBW], f32)
                    for c in range(EC):
                        nc.tensor.matmul(
                            ps,
                            lhsT=wqs[:, c, p_ * P:(p_ + 1) * P],
                            rhs=xTs[:, c, m * SBW:(m + 1) * SBW],
                            start=(c == 0), stop=(c == EC - 1),
                        )
                    nc.vector.tensor_copy(QT[p_][:, m * SBW:(m + 1) * SBW], ps)

        # KT pairs [(2 heads' d), t]
        with tc.tile_pool(name="wk", bufs=1) as wkp, \
             tc.tile_pool(name="psk", bufs=4, space="PSUM") as psk:
            wks = wkp.tile([P, EC, NP * P], f32)
            nc.sync.dma_start(out=wks, in_=wk_d.rearrange("(c p) m -> p c m", p=P))
            for p_ in range(NP):
                for m in range(SB):
                    ps = psk.tile([P, SBW], f32)
                    for c in range(EC):
                        nc.tensor.matmul(
                            ps,
                            lhsT=wks[:, c, p_ * P:(p_ + 1) * P],
                            rhs=xTs[:, c, m * SBW:(m + 1) * SBW],
                            start=(c == 0), stop=(c == EC - 1),
                        )
                    nc.vector.tensor_copy(KT[p_][:, m * SBW:(m + 1) * SBW], ps)

    # ---------------- Phase 2: causal attention ----------------
    otp = ctx.enter_context(tc.tile_pool(name="otp", bufs=1))
    OT = [otp.tile([P, T], f32, tag=f"ot{p}", name=f"ot{p}") for p in range(NP)]
    with tc.tile_pool(name="pt", bufs=3) as ptp, \
         tc.tile_pool(name="acc", bufs=2) as accp, \
         tc.tile_pool(name="rep", bufs=2) as repp, \
         tc.tile_pool(name="rsb", bufs=2) as rsbp, \
         tc.tile_pool(name="psst", bufs=2, space="PSUM") as psst, \
         tc.tile_pool(name="psot", bufs=1, space="PSUM") as psot, \
         tc.tile_pool(name="psl", bufs=1, space="PSUM") as psl, \
         tc.tile_pool(name="psrep", bufs=1, space="PSUM") as psrep:
        for p_ in range(NP):
            kt, qt, vt, oc = KT[p_], QT[p_], V[p_], OT[p_]
            for j in range(SB):
                ntt = 4 * (j + 1)
                ot_ps = psot.tile([P, SBW], f32)
                acc = accp.tile([P, 2 * SBW], f32)
                for i in range(ntt):
                    st_ps = psst.tile([P, 2 * SBW], f32)
                    # scores (transposed): two heads row-tiled, K=64 each
                    nc.tensor.matmul(
                        st_ps[:, 0:SBW],
                        lhsT=kt[0:DH, i * P:(i + 1) * P],
                        rhs=qt[0:DH, j * SBW:(j + 1) * SBW],
                        start=True, stop=True,
                    )
                    nc.tensor.matmul(
                        st_ps[:, SBW:2 * SBW],
                        lhsT=kt[DH:P, i * P:(i + 1) * P],
                        rhs=qt[DH:P, j * SBW:(j + 1) * SBW],
                        start=True, stop=True,
                    )
                    pt = ptp.tile([P, 2 * SBW], f32)
                    nc.scalar.activation(pt, st_ps, Exp, bias=0.0, scale=0.125)
                    dd = i - 4 * j
                    if dd >= 0:  # diagonal block: zero out t > s
                        nc.vector.tensor_mul(
                            pt[:, 0:SBW], pt[:, 0:SBW], masks_sb[:, dd, :])
                        nc.vector.tensor_mul(
                            pt[:, SBW:2 * SBW], pt[:, SBW:2 * SBW],
                            masks_sb[:, dd, :])
                    # softmax denominator accumulate
                    if i == 0:
                        nc.vector.tensor_copy(acc, pt)
                    else:
                        nc.vector.tensor_add(acc, acc, pt)
                    # PV: two heads col-tiled
                    nc.tensor.matmul(
                        ot_ps[0:DH, :],
                        lhsT=vt[:, i, 0:DH],
                        rhs=pt[:, 0:SBW],
                        start=(i == 0), stop=(i == ntt - 1),
                    )
                    nc.tensor.matmul(
                        ot_ps[DH:P, :],
                        lhsT=vt[:, i, DH:P],
                        rhs=pt[:, SBW:2 * SBW],
                        start=(i == 0), stop=(i == ntt - 1),
                        skip_group_check=True,
                    )
                # finalize: l = colsum(acc); OT = ot / l
                rep_ps = psrep.tile([P, SBW], f32)
                for h in range(2):
                    l_ps = psl.tile([1, SBW], f32, tag="lps")
                    nc.tensor.matmul(
                        l_ps, lhsT=ones_col,
                        rhs=acc[:, h * SBW:(h + 1) * SBW],
                        start=True, stop=True,
                    )
                    r_sb = rsbp.tile([1, SBW], f32, tag="rsb")
                    nc.vector.reciprocal(r_sb, l_ps)
                    nc.tensor.matmul(
                        rep_ps[h * DH:(h + 1) * DH, :],
                        lhsT=ones_row, rhs=r_sb,
                        start=True, stop=True,
                        skip_group_check=(h == 1),
                    )
                rep_sb = repp.tile([P, SBW], f32)
                nc.vector.tensor_copy(rep_sb, rep_ps)
                nc.vector.tensor_mul(
                    oc[:, j * SBW:(j + 1) * SBW], ot_ps, rep_sb)

    # ---------------- Phase 3: output projection (partial) ----------------
    with tc.tile_pool(name="wp", bufs=1) as wpp, \
         tc.tile_pool(name="ysb", bufs=3) as ysbp, \
         tc.tile_pool(name="psy", bufs=4, space="PSUM") as psy:
        wps = wpp.tile([P, NP, E], f32)
        nc.sync.dma_start(out=wps, in_=wpT_d.rearrange("(c p) m -> p c m", p=P))
        for st in range(T // P):
            y_sb = ysbp.tile([P, E], f32)
            for half in range(2):
                ps = psy.tile([P, SBW], f32)
                for c in range(NP):
                    nc.tensor.matmul(
                        ps,
                        lhsT=OT[c][:, st * P:(st + 1) * P],
                        rhs=wps[:, c, half * SBW:(half + 1) * SBW],
                        start=(c == 0), stop=(c == NP - 1),
                    )
                nc.vector.tensor_copy(y_sb[:, half * SBW:(half + 1) * SBW], ps)
            nc.sync.dma_start(out=y_d[st * P:(st + 1) * P, :], in_=y_sb)


def build_program():
    nc = bass.Bass("TRN2", target_bir_lowering=False, debug=False)
    xT_d = nc.declare_dram_parameter("xT", [E, T], f32, isOutput=False).ap()
    wq_d = nc.declare_dram_parameter("wq", [E, NP * P], f32, isOutput=False).ap()
    wk_d = nc.declare_dram_parameter("wk", [E, NP * P], f32, isOutput=False).ap()
    wv_d = nc.declare_dram_parameter("wv", [E, NP * P], f32, isOutput=False).ap()
    wpT_d = nc.declare_dram_parameter("wpT", [NP * P, E], f32, isOutput=False).ap()
    mask_d = nc.declare_dram_parameter("mask", [4, P, SBW], f32, isOutput=False).ap()
    y_d = nc.declare_dram_parameter("y", [T, E], f32, isOutput=True).ap()

    with tile.TileContext(nc, pool_alloc_mode="queue") as tc:
        with ExitStack() as ctx:
            _build_body(nc, tc, ctx, xT_d, wq_d, wk_d, wv_d, wpT_d, mask_d, y_d)
    _split_excess_waits(nc)
    return nc


def make_masks():
    m = np.zeros((4, P, SBW), dtype=np.float32)
    tt = np.arange(P)[:, None]
    ss = np.arange(SBW)[None, :]
    for dd in range(4):
        m[dd] = (P * dd + tt <= ss).astype(np.float32)
    return m


def make_in_maps(x, Wq, Wk, Wv, Wp):
    masks = make_masks()
    in_maps = []
    for b in range(B):
        for g in range(2):
            hs = slice(g * 8, g * 8 + 8)
            in_maps.append({
                "xT": np.ascontiguousarray(x[b].T),
                "wq": np.ascontiguousarray(
                    Wq[hs].transpose(1, 0, 2).reshape(E, 512)),
                "wk": np.ascontiguousarray(
                    Wk[hs].transpose(1, 0, 2).reshape(E, 512)),
                "wv": np.ascontiguousarray(
                    Wv[hs].transpose(1, 0, 2).reshape(E, 512)),
                "wpT": np.ascontiguousarray(Wp[:, g * 512:(g + 1) * 512].T),
                "mask": masks,
            })
    return in_maps


def kernel(x, Wq, Wk, Wv, Wp, bp):
    global _PROG
    x = np.asarray(x, dtype=np.float32)
    Wq = np.asarray(Wq, dtype=np.float32)
    Wk = np.asarray(Wk, dtype=np.float32)
    Wv = np.asarray(Wv, dtype=np.float32)
    Wp = np.asarray(Wp, dtype=np.float32)
    bp = np.asarray(bp, dtype=np.float32)

    if _PROG is None:
        _PROG = build_program()
    nc = _PROG

    in_maps = make_in_maps(x, Wq, Wk, Wv, Wp)
    res = run_bass_kernel_spmd(nc, in_maps, list(range(NCORES)))
    LAST["res"] = res
    LAST["exec_time_ns"] = res.exec_time_ns

    ys = [res.results[i]["y"] for i in range(NCORES)]
    out = np.stack([ys[2 * b] + ys[2 * b + 1] for b in range(B)], axis=0)
    out += bp[None, None, :]
    return out.astype(np.float32)


# revision 8
# speedup vs baseline: 2.1803x; 2.1803x over previous
"""Multi-head causal attention (B=4, T=2048, E=1024, H=16) on 8 NeuronCores.

Sharding: core = (batch b, head-group g of 8 heads). Each core computes its
heads' attention + a partial output projection; host sums the two partials
per batch and adds the bias (the "all-reduce" of the tensor-parallel plan).

Device layout (per core):
  xT  [E, T]   x[b] transposed on host; e on partitions (8 chunks of 128).
  wq/wk/wv [E, 512]: this core's 8 heads' weights, columns = h_local*64+d.
  wpT [512, E]: Wp rows for this core's 512 e-dims, transposed on host.
  QT/KT pair tiles [(2 heads' d)=128, T]; V tiles [t, 193] carry both heads'
  values plus ones/zeros columns so the PV matmuls emit softmax denominators
  into spare PSUM rows:
    cols 0:64    V_h0   -> ota rows 0:64   = OT_h0
    col  64      ones   -> ota row 64      = l_h0
    cols 65:97,98:129 zeros -> otb rows 0:31,33:63 = 0
    col  97      ones   -> otb row 32      = l_h1
    cols 129:193 V_h1   -> otb rows 64:128 = OT_h1 (lands on right partitions)
  Scores are computed transposed ST[t, s] (two heads' K=64 matmuls), exp on
  ACT with the 1/8 scale folded in, causality via reduced-width matmuls plus
  one [128,128] triangle mask on the diagonal subtile. Reciprocal runs on 2
  partitions (cost scales with free-size, not lanes), then a step-0 free-dim
  DMA replicates it across partitions for the normalize multiply. The
  normalized OT[(h,d), s] is exactly the lhsT the output projection needs.

FAST mode (default) tags matmul operands float32r (tf32-like, 11-bit
mantissa, 1 PE cycle/row); PRECISE mode keeps everything fp32 (4 cycles/row).
Set BASS_MHA_PRECISE=1 to force the exact variant.
"""

import os
import sys
from contextlib import ExitStack

import numpy as np

sys.path.insert(0, "/opt/trn_rl_repo")

import concourse.bass as bass
import concourse.tile as tile
from concourse import mybir
from concourse.bass_utils import run_bass_kernel_spmd

f32 = mybir.dt.float32
f32r = mybir.dt.float32r

B, T, E, H = 4, 2048, 1024, 16
DH = E // H          # 64
P = 128              # partitions
EC = E // P          # 8 e-chunks
NP = 4               # head pairs per core (8 heads)
SBW = 512            # s-block width
SB = T // SBW        # 4 s-blocks
TT = T // P          # 16 t-tiles
VW = 193             # V tile width: V0|1|zeros|1|V1
NCORES = 8

_PROGS = {}
LAST = {}


def _split_excess_waits(nc, limit=1):
    """walrus in this container encodes at most one sync-wait per
    instruction; move extras onto same-engine NOPs placed just before."""
    for fn in nc.m.functions:
        for bb in fn.blocks:
            out = []
            changed = False
            for inst in bb.instructions:
                si = inst.sync_info
                if si is not None and si.on_wait and len(si.on_wait) > limit:
                    waits = list(si.on_wait)
                    extra, keep = waits[:-limit], waits[-limit:]
                    for k, w in enumerate(extra):
                        nop = mybir.InstNoOp(
                            name=f"{inst.name}-wsplit{k}", ins=[], outs=[]
                        )
                        nop.engine = inst.engine
                        nop.sync_info = type(si)(on_wait=[w], on_update=[])
                        nc.register_instruction(nop)
                        out.append(nop)
                    si.on_wait = keep
                    changed = True
                out.append(inst)
            if changed:
                bb.instructions = out
    return nc


def _replicate_row_ap(src_row):
    """AP reading one SBUF partition row [1, W] as [1, 64, W] via a step-0
    free dim — DMA'd to a [64, W] destination this replicates the row."""
    return bass.AP(
        tensor=src_row.tensor,
        offset=src_row.offset,
        ap=[list(src_row.ap[0]), [0, 64], list(src_row.ap[1])],
    )


def _build_body(nc, tc, ctx, DT, xT_d, wq_d, wk_d, wv_d, wpT_d, tri_d, y_d):
    Exp = mybir.ActivationFunctionType.Exp
    Ln = mybir.ActivationFunctionType.Ln

    persist = ctx.enter_context(tc.tile_pool(name="persist", bufs=1))
    QT = [persist.tile([P, T], DT, tag=f"qt{p}", name=f"qt{p}") for p in range(NP)]
    KT = [persist.tile([P, T], DT, tag=f"kt{p}", name=f"kt{p}") for p in range(NP)]
    V = [persist.tile([P, TT, VW], DT, tag=f"v{p}", name=f"v{p}") for p in range(NP)]
    tri_sb = persist.tile([P, P], DT, tag="tri")
    nc.sync.dma_start(out=tri_sb, in_=tri_d)

    # ---------------- Phase 1: QKV projections ----------------
    with tc.tile_pool(name="xp", bufs=1) as xp:
        xTs = xp.tile([P, EC, T], DT)
        for c in range(EC):
            nc.sync.dma_start(out=xTs[:, c, :], in_=xT_d[c * P:(c + 1) * P, :])

        # V natural for all 4 pairs at once (N=512), scattered into V tiles
        with tc.tile_pool(name="wv", bufs=1) as wvp, \
             tc.tile_pool(name="psv", bufs=4, space="PSUM") as psv:
            wvs = wvp.tile([P, EC, NP * P], DT)
            nc.sync.dma_start(out=wvs, in_=wv_d.rearrange("(c p) m -> p c m", p=P))
            for p_ in range(NP):
                nc.vector.memset(V[p_][:, :, 64:65].bitcast(f32), 1.0)
                nc.vector.memset(V[p_][:, :, 65:97].bitcast(f32), 0.0)
                nc.vector.memset(V[p_][:, :, 97:98].bitcast(f32), 1.0)
                nc.vector.memset(V[p_][:, :, 98:129].bitcast(f32), 0.0)
            for tt in range(TT):
                ps = psv.tile([P, NP * P], f32)
                for c in range(EC):
                    nc.tensor.matmul(
                        ps,
                        lhsT=xTs[:, c, tt * P:(tt + 1) * P],
                        rhs=wvs[:, c, :],
                        start=(c == 0), stop=(c == EC - 1),
                    )
                for p_ in range(NP):
                    nc.vector.tensor_copy(
                        V[p_][:, tt, 0:64], ps[:, p_ * P:p_ * P + 64])
                    nc.vector.tensor_copy(
                        V[p_][:, tt, 129:193], ps[:, p_ * P + 64:(p_ + 1) * P])

        # QT pairs [(2 heads' d), s]
        with tc.tile_pool(name="wq", bufs=1) as wqp, \
             tc.tile_pool(name="psq", bufs=4, space="PSUM") as psq:
            wqs = wqp.tile([P, EC, NP * P], DT)
            nc.sync.dma_start(out=wqs, in_=wq_d.rearrange("(c p) m -> p c m", p=P))
            for p_ in range(NP):
                for m in range(SB):
                    ps = psq.tile([P, SBW], f32)
                    for c in range(EC):
                        nc.tensor.matmul(
                            ps,
                            lhsT=wqs[:, c, p_ * P:(p_ + 1) * P],
                            rhs=xTs[:, c, m * SBW:(m + 1) * SBW],
                            start=(c == 0), stop=(c == EC - 1),
                        )
                    nc.vector.tensor_copy(QT[p_][:, m * SBW:(m + 1) * SBW], ps)

        # KT pairs [(2 heads' d), t]
        with tc.tile_pool(name="wk", bufs=1) as wkp, \
             tc.tile_pool(name="psk", bufs=4, space="PSUM") as psk:
            wks = wkp.tile([P, EC, NP * P], DT)
            nc.sync.dma_start(out=wks, in_=wk_d.rearrange("(c p) m -> p c m", p=P))
            for p_ in range(NP):
                for m in range(SB):
                    ps = psk.tile([P, SBW], f32)
                    for c in range(EC):
                        nc.tensor.matmul(
                            ps,
                            lhsT=wks[:, c, p_ * P:(p_ + 1) * P],
                            rhs=xTs[:, c, m * SBW:(m + 1) * SBW],
                            start=(c == 0), stop=(c == EC - 1),
                        )
                    nc.vector.tensor_copy(KT[p_][:, m * SBW:(m + 1) * SBW], ps)

    # ---------------- Phase 2: causal attention ----------------
    otp = ctx.enter_context(tc.tile_pool(name="otp", bufs=1))
    OT = [otp.tile([P, T], DT, tag=f"ot{p}", name=f"ot{p}") for p in range(NP)]
    with tc.tile_pool(name="pt", bufs=3) as ptp, \
         tc.tile_pool(name="lsb", bufs=2) as lsp, \
         tc.tile_pool(name="rsb", bufs=2) as rsp, \
         tc.tile_pool(name="rep", bufs=2) as repp, \
         tc.tile_pool(name="psst", bufs=2, space="PSUM") as psst, \
         tc.tile_pool(name="psota", bufs=2, space="PSUM") as psota, \
         tc.tile_pool(name="psotb", bufs=2, space="PSUM") as psotb:
        for p_ in range(NP):
            kt, qt, vt, oc = KT[p_], QT[p_], V[p_], OT[p_]
            for j in range(SB):
                ntt = 4 * (j + 1)
                ota = psota.tile([P, SBW], f32)
                otb = psotb.tile([P, SBW], f32)
                for i in range(ntt):
                    dd = i - 4 * j
                    s_lo = P * dd if dd >= 0 else 0
                    st = psst.tile([P, 2 * SBW], f32)
                    nc.tensor.matmul(
                        st[:, s_lo:SBW],
                        lhsT=kt[0:DH, i * P:(i + 1) * P],
                        rhs=qt[0:DH, j * SBW + s_lo:(j + 1) * SBW],
                        start=True, stop=True,
                    )
                    nc.tensor.matmul(
                        st[:, SBW + s_lo:2 * SBW],
                        lhsT=kt[DH:P, i * P:(i + 1) * P],
                        rhs=qt[DH:P, j * SBW + s_lo:(j + 1) * SBW],
                        start=True, stop=True,
                    )
                    pt = ptp.tile([P, 2 * SBW], DT)
                    st3 = st.rearrange("p (h w) -> p h w", h=2)[:, :, s_lo:SBW]
                    pt3 = pt.rearrange("p (h w) -> p h w", h=2)[:, :, s_lo:SBW]
                    nc.scalar.activation(pt3, st3, Exp, bias=0.0, scale=0.125)
                    if dd >= 0:
                        nc.vector.tensor_mul(
                            pt[:, s_lo:s_lo + P], pt[:, s_lo:s_lo + P], tri_sb)
                        nc.vector.tensor_mul(
                            pt[:, SBW + s_lo:SBW + s_lo + P],
                            pt[:, SBW + s_lo:SBW + s_lo + P], tri_sb)
                    nc.tensor.matmul(
                        ota[0:65, s_lo:SBW],
                        lhsT=vt[:, i, 0:65],
                        rhs=pt[:, s_lo:SBW],
                        start=(i == 0), stop=(i == ntt - 1),
                    )
                    nc.tensor.matmul(
                        otb[:, s_lo:SBW],
                        lhsT=vt[:, i, 65:VW],
                        rhs=pt[:, SBW + s_lo:2 * SBW],
                        start=(i == 0), stop=(i == ntt - 1),
                    )
                # finalize: l_h0 = ota row 64, l_h1 = otb row 32
                r_sb = rsp.tile([P, SBW], f32)
                if DT is f32r:
                    # 1/l = exp(-ln(l)) on ACT: free-size-bound, ~0.4us/op
                    ln_sb = lsp.tile([P, SBW], f32)
                    nc.scalar.activation(ln_sb[64:65, :], ota[64:65, :], Ln)
                    nc.scalar.activation(ln_sb[32:33, :], otb[32:33, :], Ln)
                    nc.scalar.activation(r_sb[64:65, :], ln_sb[64:65, :],
                                         Exp, bias=0.0, scale=-1.0)
                    nc.scalar.activation(r_sb[32:33, :], ln_sb[32:33, :],
                                         Exp, bias=0.0, scale=-1.0)
                else:
                    nc.vector.reciprocal(r_sb[64:65, :], ota[64:65, :])
                    nc.vector.reciprocal(r_sb[32:33, :], otb[32:33, :])
                rep = repp.tile([P, SBW], f32)
                nc.sync.dma_start(
                    out=rep[0:64, :], in_=_replicate_row_ap(r_sb[64:65, :]))
                nc.sync.dma_start(
                    out=rep[64:128, :], in_=_replicate_row_ap(r_sb[32:33, :]))
                jblk = slice(j * SBW, (j + 1) * SBW)
                nc.vector.tensor_mul(
                    oc[0:64, jblk], ota[0:64, :], rep[0:64, :])
                nc.vector.tensor_mul(
                    oc[64:128, jblk], otb[64:128, :], rep[64:128, :])

    # ---------------- Phase 3: output projection (partial) ----------------
    with tc.tile_pool(name="wp", bufs=1) as wpp, \
         tc.tile_pool(name="ysb", bufs=3) as ysbp, \
         tc.tile_pool(name="psy", bufs=4, space="PSUM") as psy:
        wps = wpp.tile([P, NP, E], DT)
        nc.sync.dma_start(out=wps, in_=wpT_d.rearrange("(c p) m -> p c m", p=P))
        for st_ in range(T // P):
            y_sb = ysbp.tile([P, E], f32)
            for half in range(2):
                ps = psy.tile([P, SBW], f32)
                for c in range(NP):
                    nc.tensor.matmul(
                        ps,
                        lhsT=OT[c][:, st_ * P:(st_ + 1) * P],
                        rhs=wps[:, c, half * SBW:(half + 1) * SBW],
                        start=(c == 0), stop=(c == NP - 1),
                    )
                nc.vector.tensor_copy(y_sb[:, half * SBW:(half + 1) * SBW], ps)
            nc.sync.dma_start(out=y_d[st_ * P:(st_ + 1) * P, :], in_=y_sb)


def build_program(fast=True):
    DT = f32r if fast else f32
    nc = bass.Bass("TRN2", target_bir_lowering=False, debug=False)
    xT_d = nc.declare_dram_parameter("xT", [E, T], DT, isOutput=False).ap()
    wq_d = nc.declare_dram_parameter("wq", [E, NP * P], DT, isOutput=False).ap()
    wk_d = nc.declare_dram_parameter("wk", [E, NP * P], DT, isOutput=False).ap()
    wv_d = nc.declare_dram_parameter("wv", [E, NP * P], DT, isOutput=False).ap()
    wpT_d = nc.declare_dram_parameter("wpT", [NP * P, E], DT, isOutput=False).ap()
    tri_d = nc.declare_dram_parameter("tri", [P, P], DT, isOutput=False).ap()
    y_d = nc.declare_dram_parameter("y", [T, E], f32, isOutput=True).ap()

    with tile.TileContext(nc, pool_alloc_mode="queue") as tc:
        with ExitStack() as ctx:
            _build_body(nc, tc, ctx, DT, xT_d, wq_d, wk_d, wv_d, wpT_d,
                        tri_d, y_d)
    _split_excess_waits(nc)
    return nc


def make_tri():
    tt = np.arange(P)[:, None]
    ss = np.arange(P)[None, :]
    return (tt <= ss).astype(np.float32)


def make_in_maps(x, Wq, Wk, Wv, Wp):
    tri = make_tri()
    in_maps = []
    for b in range(B):
        for g in range(2):
            hs = slice(g * 8, g * 8 + 8)
            in_maps.append({
                "xT": np.ascontiguousarray(x[b].T),
                "wq": np.ascontiguousarray(
                    Wq[hs].transpose(1, 0, 2).reshape(E, 512)),
                "wk": np.ascontiguousarray(
                    Wk[hs].transpose(1, 0, 2).reshape(E, 512)),
                "wv": np.ascontiguousarray(
                    Wv[hs].transpose(1, 0, 2).reshape(E, 512)),
                "wpT": np.ascontiguousarray(Wp[:, g * 512:(g + 1) * 512].T),
                "tri": tri,
            })
    return in_maps


def kernel(x, Wq, Wk, Wv, Wp, bp):
    x = np.asarray(x, dtype=np.float32)
    Wq = np.asarray(Wq, dtype=np.float32)
    Wk = np.asarray(Wk, dtype=np.float32)
    Wv = np.asarray(Wv, dtype=np.float32)
    Wp = np.asarray(Wp, dtype=np.float32)
    bp = np.asarray(bp, dtype=np.float32)

    fast = os.environ.get("BASS_MHA_PRECISE", "0") != "1"
    if fast not in _PROGS:
        _PROGS[fast] = build_program(fast=fast)
    nc = _PROGS[fast]

    in_maps = make_in_maps(x, Wq, Wk, Wv, Wp)
    res = run_bass_kernel_spmd(nc, in_maps, list(range(NCORES)))
    LAST["res"] = res
    LAST["exec_time_ns"] = res.exec_time_ns

    ys = [res.results[i]["y"] for i in range(NCORES)]
    out = np.stack([ys[2 * b] + ys[2 * b + 1] for b in range(B)], axis=0)
    out += bp[None, None, :]
    return out.astype(np.float32)
